# revision 38
# baseline (speedup 1.0000x reference)
"""Trainium2 Bass kernel for nn_AttentionSACModel (sparse_attention).

Data-parallel across 8 NeuronCores: obs sharded along batch, params replicated.
Feature-major on-device layout (batch on the matmul free dim).

v2 structure (per 512-batch tile, per core):
  - z embeds: 2x row-tiled K=32 matmul pairs (x packed 2 interactors per
    64-partition slab), prelu batched over 2 PSUM banks.
  - energy: PK = Wk z + Wq own (2-pass accumulate), tanh batched over 2 banks.
  - scores: Va col-tiled 4x into one PSUM bank, exp per 4 interactors.
  - combine uses ctx = Wv (sum_n e_n z_n) * (1/denom) -- Wv applied once
    (blockdiag commutes with the per-head softmax weighting).
  - e-broadcast: row-tiled K=32 Eb matmul pairs; multiply paired [128,1024];
    sum over n via bf16 2x vector tree.
  - denom: Ds accumulate, reciprocal_approx_fast, Rb broadcast.
  - head MLP in bf16 weights.
"""
import sys
import os

if "/opt/trn_rl_repo" not in sys.path:
    sys.path.insert(0, "/opt/trn_rl_repo")

import numpy as np
import ml_dtypes
_bf16np = ml_dtypes.bfloat16

OWN_DIM = 7
INT_DIM = 7
N_INTR = 20
H = 3
D = 42
TOT = H * D            # 126
ATTN = 128
HID = 256
NOUT = 4
B = 32768
N_CORES = 8
BC = B // N_CORES      # 4096 rows per core
NB = 512               # batch tile (matmul free dim)
NT = BC // NB          # 8 tiles per core
ALPHA = 0.2            # leaky relu slope
NPAIR = 11             # 10 interactor pairs + (own, zero)
ZPRELU_V = (8, 9)  # z-pairs whose prelu runs off-scalar

_BUILT = {}


def _build_nc():
    import concourse.bacc as bacc
    import concourse.bass as bass
    import concourse.tile as tile
    from concourse import mybir

    f32 = mybir.dt.float32
    f32r = mybir.dt.float32r
    bf16 = mybir.dt.bfloat16
    AF = mybir.ActivationFunctionType
    ALU = mybir.AluOpType

    nc = bacc.Bacc()

    # ---- DRAM I/O ----
    xp_d = nc.dram_tensor("xp", [64, NPAIR, BC], bf16, kind="ExternalInput")
    w2_d = nc.dram_tensor("w2", [64, 128], bf16, kind="ExternalInput")     # W_int at rows 0-6 and 32-38
    wn2_d = nc.dram_tensor("wn2", [64, 128], bf16, kind="ExternalInput")   # W_own at rows 0-6
    wq_d = nc.dram_tensor("wqb", [128, 128], bf16, kind="ExternalInput")
    wk_d = nc.dram_tensor("wkb", [128, 128], bf16, kind="ExternalInput")
    wv_d = nc.dram_tensor("wvb", [126, 128], bf16, kind="ExternalInput")
    va_d = nc.dram_tensor("va32", [128, 32], bf16, kind="ExternalInput")
    eb_d = nc.dram_tensor("ebt", [128, 128], bf16, kind="ExternalInput")   # broadcast selector
    ds_d = nc.dram_tensor("densel", [128, 3], bf16, kind="ExternalInput")
    rb_d = nc.dram_tensor("rbc", [3, 128], bf16, kind="ExternalInput")
    wat_d = nc.dram_tensor("wat", [128, 128], bf16, kind="ExternalInput")
    wop_d = nc.dram_tensor("wop", [128, 128], bf16, kind="ExternalInput")
    wh1_d = nc.dram_tensor("wh1r", [128, 512], bf16, kind="ExternalInput")  # [p, kc*256+m]
    wh2_d = nc.dram_tensor("wh2r", [128, 512], bf16, kind="ExternalInput")
    wout_d = nc.dram_tensor("woutr", [128, 8], bf16, kind="ExternalInput")  # [p, kc*4+m]
    bown_d = nc.dram_tensor("bown", [128, 1], f32, kind="ExternalInput")
    bint_d = nc.dram_tensor("bint", [128, 1], f32, kind="ExternalInput")
    bat_d = nc.dram_tensor("bat", [128, 1], f32, kind="ExternalInput")
    bop_d = nc.dram_tensor("bop", [128, 1], f32, kind="ExternalInput")
    bh1_d = nc.dram_tensor("bh1", [128, 2], f32, kind="ExternalInput")
    bh2_d = nc.dram_tensor("bh2", [128, 2], f32, kind="ExternalInput")
    bout_d = nc.dram_tensor("bout", [4, 1], f32, kind="ExternalInput")
    out_d = nc.dram_tensor("outT", [NOUT, BC], f32, kind="ExternalOutput")

    with tile.TileContext(nc) as tc:
        with tc.tile_pool(name="const", bufs=1) as cst, \
             tc.tile_pool(name="px", bufs=3) as px, \
             tc.tile_pool(name="pzt", bufs=20) as pzt, \
             tc.tile_pool(name="pen", bufs=6) as pen, \
             tc.tile_pool(name="peg", bufs=12) as peg, \
             tc.tile_pool(name="ppn", bufs=21) as ppn, \
             tc.tile_pool(name="ptt", bufs=2) as ptt, \
             tc.tile_pool(name="ph", bufs=2) as ph, \
             tc.tile_pool(name="pwork", bufs=2, space="PSUM") as pwork, \
             tc.tile_pool(name="pps", bufs=1, space="PSUM") as pps, \
             tc.tile_pool(name="ppe", bufs=2, space="PSUM") as ppe, \
             tc.tile_pool(name="paux", bufs=1, space="PSUM") as paux:

            # ---- load constants ----
            W2 = cst.tile([64, 128], bf16)
            WN2 = cst.tile([64, 128], bf16)
            Wq = cst.tile([128, 128], bf16)
            Wk = cst.tile([128, 128], bf16)
            Wv = cst.tile([126, 128], bf16)
            Va = cst.tile([128, 32], bf16)
            EbT = cst.tile([128, 128], bf16)
            Ds = cst.tile([128, 3], bf16)
            Rb = cst.tile([3, 128], bf16)
            Wat = cst.tile([128, 128], bf16)
            Wop = cst.tile([128, 128], bf16)
            WH1 = cst.tile([128, 512], bf16)
            WH2 = cst.tile([128, 512], bf16)
            WOUT = cst.tile([128, 8], bf16)
            Bown = cst.tile([128, 1], f32)
            Bint = cst.tile([128, 1], f32)
            Bat = cst.tile([128, 1], f32)
            Bop = cst.tile([128, 1], f32)
            BH1 = cst.tile([128, 2], f32)
            BH2 = cst.tile([128, 2], f32)
            Bout = cst.tile([4, 1], f32)
            for t_sb, t_dr in [(W2, w2_d), (WN2, wn2_d), (Wk, wk_d),
                               (Wq, wq_d), (Bown, bown_d), (Bint, bint_d)]:
                nc.sync.dma_start(out=t_sb, in_=t_dr[:, :])

            def load_late_consts():
                for t_sb, t_dr in [(Va, va_d), (EbT, eb_d), (Ds, ds_d),
                                   (Rb, rb_d), (Wv, wv_d), (Wat, wat_d),
                                   (Wop, wop_d), (WH1, wh1_d), (WH2, wh2_d),
                                   (WOUT, wout_d), (Bat, bat_d), (Bop, bop_d),
                                   (BH1, bh1_d), (BH2, bh2_d), (Bout, bout_d)]:
                    nc.scalar.dma_start(out=t_sb, in_=t_dr[:, :])

            with nc.allow_low_precision(reason="bf16 intermediates; accums f32"):
                state = {}

                def load_x(t):
                    bs = t * NB
                    XP = px.tile([64, NPAIR, NB], bf16, tag="xp", name="XP")
                    nc.sync.dma_start(out=XP, in_=xp_d[:, :, bs:bs + NB])
                    state[t] = {"XP": XP, "ZT": [], "EG": []}

                def z_pair(t, m):
                    """embed pair m: 2x row-tiled K=32 matmuls + paired prelu"""
                    st = state[t]
                    XP = st["XP"]
                    PZ = pwork.tile([128, 2, NB], f32, tag="work", name="PZ")
                    lhs = WN2 if m == 10 else W2
                    nc.tensor.matmul(PZ[:, 0, :], lhs[0:32, :], XP[0:32, m, :],
                                     tile_position=(0, 0))
                    if m < 10:
                        nc.tensor.matmul(PZ[:, 1, :], lhs[32:64, :],
                                         XP[32:64, m, :], tile_position=(32, 0))
                    if m == 10:
                        OWN = ph.tile([128, NB], bf16, tag="own", name="OWN", bufs=4)
                        nc.scalar.activation(OWN, PZ[:, 0, :], AF.Prelu,
                                             alpha=ALPHA)
                        st["OWN"] = OWN
                    else:
                        ZT = pzt.tile([128, 2, NB], bf16, tag="zt", name="ZT")
                        if m in ZPRELU_V:
                            ZC = ptt.tile([128, 2, NB], bf16, tag="zc",
                                          name="ZC", bufs=3)
                            nc.vector.tensor_copy(ZC, PZ)
                            nc.vector.scalar_tensor_tensor(
                                out=ZT, in0=ZC, scalar=ALPHA, in1=ZC,
                                op0=ALU.mult, op1=ALU.max)
                        else:
                            nc.scalar.activation(ZT, PZ, AF.Prelu, alpha=ALPHA)
                        st["ZT"].append(ZT)

                def attn_energy(t, m):
                    """energy tanh for interactors 2m, 2m+1 of tile t"""
                    st = state[t]
                    OWN = st["OWN"]
                    PK = pwork.tile([128, 2, NB], f32, tag="work", name="PK")
                    ZT = st["ZT"][m]
                    # group same-weight matmuls so LDWEIGHTS amortizes
                    nc.tensor.matmul(PK[:, 0, :], Wk, ZT[:, 0, :],
                                     start=True, stop=False)
                    nc.tensor.matmul(PK[:, 1, :], Wk, ZT[:, 1, :],
                                     start=True, stop=False)
                    nc.tensor.matmul(PK[:, 0, :], Wq, OWN,
                                     start=False, stop=True)
                    nc.tensor.matmul(PK[:, 1, :], Wq, OWN,
                                     start=False, stop=True)
                    EN = pen.tile([128, 2, NB], bf16, tag="en", name="EN")
                    nc.scalar.activation(EN, PK, AF.Tanh)
                    st.setdefault("EN", []).append(EN)

                def attn_score(t, m):
                    """score matmul + exp for pair m (lagged so Va never
                    waits on the tanh)"""
                    st = state[t]
                    EN = st["EN"][m]
                    jj = (2 * m) % 4
                    if jj == 0:
                        st["PS"] = pps.tile([128, NB], f32, tag="ps", name="PS")
                    PS = st["PS"]
                    for i in range(2):
                        j = jj + i
                        nc.tensor.matmul(PS[32 * j:32 * (j + 1), :], Va, EN[:, i, :],
                                         tile_position=(0, 32 * j))
                    if jj == 2:
                        EG = peg.tile([128, NB], bf16, tag="eg", name="EG")
                        nc.scalar.activation(EG, PS, AF.Exp)
                        st["EG"].append(EG)

                def combine_one(t, n):
                    """broadcast exp (Eb matmul) + multiply with z"""
                    st = state[t]
                    g, j = n // 4, n % 4
                    EG = st["EG"][g]
                    PE1 = ppe.tile([128, NB], f32, tag="pe", name="PE1")
                    nc.tensor.matmul(PE1, EbT[32 * j:32 * (j + 1), :],
                                     EG[32 * j:32 * (j + 1), :],
                                     tile_position=(32 * j, 0))
                    PN = ppn.tile([126, NB], bf16, tag="pn", name="PN")
                    nc.vector.tensor_tensor(out=PN, in0=PE1[0:126, :],
                                            in1=st["ZT"][n // 2][0:126, n % 2, :],
                                            op=ALU.mult)
                    st.setdefault("PN", []).append(PN)

                def tt_add(st, key, a, b, tag, eng):
                    Tl = ptt.tile([126, NB], bf16, tag=tag, name=tag)
                    eng.tensor_tensor(out=Tl, in0=a, in1=b, op=ALU.add)
                    st[key] = Tl
                    return Tl

                def tree_l1(t, i):
                    """pair-add PN[2i] + PN[2i+1]; gpsimd helps (idle engine)"""
                    st = state[t]
                    PNs = st["PN"]
                    eng = nc.gpsimd if i % 2 == 0 else nc.vector
                    tt_add(st, f"T1_{i}", PNs[2 * i], PNs[2 * i + 1],
                           f"t1_{i}", eng)

                def tree_l2(t, j):
                    """T2_j = T1_{2j} + T1_{2j+1} (j = 0..4)"""
                    st = state[t]
                    eng = nc.gpsimd if j % 2 == 1 else nc.vector
                    tt_add(st, f"T2_{j}", st[f"T1_{2 * j}"],
                           st[f"T1_{2 * j + 1}"], f"t2_{j}", eng)

                def tree_finish(t, step):
                    st = state[t]
                    if step == 0:
                        tt_add(st, "T3A", st["T2_0"], st["T2_1"], "t3a",
                               nc.vector)
                    elif step == 1:
                        tt_add(st, "T3B", st["T2_2"], st["T2_3"], "t3b",
                               nc.gpsimd)
                    elif step == 2:
                        tt_add(st, "T4", st["T3A"], st["T3B"], "t4", nc.vector)
                    else:
                        U = ph.tile([126, NB], bf16, tag="u", name="U")
                        nc.vector.tensor_tensor(out=U, in0=st["T4"],
                                                in1=st["T2_4"], op=ALU.add)
                        st["U"] = U

                def denom_pd(t):
                    """softmax denominator -> broadcast reciprocal (no U dep)"""
                    st = state[t]
                    PD = paux.tile([128, NB], f32, tag="aux", name="PD")
                    for g in range(5):
                        nc.tensor.matmul(PD[0:3, :], Ds, st["EG"][g],
                                         start=(g == 0), stop=(g == 4))
                    RD = ph.tile([3, NB], f32, tag="rd", name="RD")
                    nc.vector.reciprocal_approx_fast(out=RD, in_=PD[0:3, :])
                    RDb = ph.tile([3, NB], bf16, tag="rdb", name="RDb")
                    nc.vector.tensor_copy(RDb, RD)
                    PR = paux.tile([128, NB], f32, tag="aux", name="PR")
                    nc.tensor.matmul(PR, Rb, RDb)
                    PRs = ph.tile([128, NB], bf16, tag="prs", name="PRs")
                    nc.vector.tensor_copy(PRs, PR)
                    st["PRs"] = PRs

                def denom_ctx(t):
                    """ctx = (Wv U) * (1/denom)"""
                    st = state[t]
                    PV = paux.tile([128, NB], f32, tag="aux", name="PV")
                    nc.tensor.matmul(PV, Wv, st["U"])
                    CTX = ph.tile([128, NB], bf16, tag="ctx", name="CTX")
                    nc.vector.tensor_tensor(out=CTX, in0=PV,
                                            in1=st["PRs"], op=ALU.mult)
                    st["CTX"] = CTX

                def head_steps(t):
                    """head MLP + output for tile t, as interleavable steps"""
                    bs = t * NB
                    h = {}

                    def s1():
                        PH1 = paux.tile([128, NB], f32, tag="aux", name="PH1")
                        nc.tensor.matmul(PH1, Wat, state[t]["CTX"])
                        h["ATT"] = ph.tile([128, NB], bf16, tag="att", name="ATT")
                        nc.scalar.activation(h["ATT"], PH1, AF.Tanh, bias=Bat)

                    def s2():
                        PH2 = paux.tile([128, NB], f32, tag="aux", name="PH2")
                        nc.tensor.matmul(PH2, Wop, state[t]["OWN"])
                        h["OWV"] = ph.tile([128, NB], bf16, tag="owv", name="OWV")
                        nc.scalar.activation(h["OWV"], PH2, AF.Tanh, bias=Bop)

                    def mk_h1(mh):
                        def s():
                            PHh = paux.tile([128, NB], f32, tag="aux", name="PHh")
                            nc.tensor.matmul(PHh, WH1[:, mh * 128:(mh + 1) * 128],
                                             h["OWV"], start=True, stop=False)
                            nc.tensor.matmul(PHh,
                                             WH1[:, 256 + mh * 128:256 + (mh + 1) * 128],
                                             h["ATT"], start=False, stop=True)
                            h[f"H1{mh}"] = ph.tile([128, NB], bf16,
                                                   tag=f"h1a{mh}", name="H1A")
                            nc.scalar.activation(h[f"H1{mh}"], PHh, AF.Prelu,
                                                 bias=BH1[:, mh:mh + 1], alpha=ALPHA)
                        return s

                    def mk_h2(mh):
                        def s():
                            PHh2 = paux.tile([128, NB], f32, tag="aux", name="PHh2")
                            nc.tensor.matmul(PHh2, WH2[:, mh * 128:(mh + 1) * 128],
                                             h["H10"], start=True, stop=False)
                            nc.tensor.matmul(PHh2,
                                             WH2[:, 256 + mh * 128:256 + (mh + 1) * 128],
                                             h["H11"], start=False, stop=True)
                            h[f"H2{mh}"] = ph.tile([128, NB], bf16,
                                                   tag=f"h2a{mh}", name="H2A")
                            nc.scalar.activation(h[f"H2{mh}"], PHh2, AF.Prelu,
                                                 bias=BH2[:, mh:mh + 1], alpha=ALPHA)
                        return s

                    def s7():
                        PO4 = paux.tile([128, NB], f32, tag="aux", name="PO4")
                        nc.tensor.matmul(PO4[0:4, :], WOUT[:, 0:4], h["H20"],
                                         start=True, stop=False)
                        nc.tensor.matmul(PO4[0:4, :], WOUT[:, 4:8], h["H21"],
                                         start=False, stop=True)
                        OT = ph.tile([4, NB], f32, tag="ot", name="OT")
                        nc.scalar.activation(OT, PO4[0:4, :], AF.Identity,
                                             bias=Bout)
                        nc.sync.dma_start(out=out_d[:, bs:bs + NB], in_=OT)
                        del state[t]

                    return [s1, s2, mk_h1(0), mk_h1(1), mk_h2(0), mk_h2(1), s7]

                def tile_body(t):
                    """z+attn for tile t, combine for t-1, head for t-2,
                    interleaved for engine-queue balance."""
                    prev = t - 1 if 0 <= t - 1 < NT else None
                    heads = head_steps(t - 2) if 0 <= t - 2 < NT else None
                    hi = 0

                    def do_head():
                        nonlocal hi
                        if heads is not None and hi < len(heads):
                            heads[hi]()
                            hi += 1

                    if t < NT and t + 1 < NT:
                        load_x(t + 1)
                    # fully merged schedule: every iteration feeds all four
                    # engines (z matmul + prelu for t, energy/score for t
                    # lagged, e-broadcast+mult+tree for t-1, head t-2)
                    for i in range(15):
                        if t < NT:
                            if i == 0:
                                z_pair(t, 10)        # own embed first
                            elif i <= 10:
                                z_pair(t, i - 1)
                        if prev is not None and i < 10:
                            combine_one(prev, 2 * i)
                            combine_one(prev, 2 * i + 1)
                        if t < NT and 4 <= i <= 13:
                            attn_energy(t, i - 4)
                        if t < NT and 5 <= i <= 14:
                            attn_score(t, i - 5)
                        if prev is not None:
                            if i == 1:
                                denom_pd(prev)
                            if i < 10:
                                tree_l1(prev, i)
                            if i in (3, 5, 7, 9, 11):
                                tree_l2(prev, (i - 3) // 2)
                            if i == 6:
                                tree_finish(prev, 0)
                            if i == 10:
                                tree_finish(prev, 1)
                            if i == 11:
                                tree_finish(prev, 2)
                            if i == 12:
                                tree_finish(prev, 3)
                                denom_ctx(prev)
                        if i in (1, 3, 5, 7, 9, 13, 14):
                            do_head()
                    while heads is not None and hi < len(heads):
                        do_head()

                # software pipeline: t does z+attn; t-1 combine; t-2 head
                load_late_consts()
                load_x(0)
                for t in range(0, NT + 2):
                    tile_body(t)

    nc.compile()
    return nc


def _host_prep(inputs):
    """Build per-core input maps (numpy only)."""
    obs = np.ascontiguousarray(inputs["obs"], dtype=np.float32)
    w_own = np.asarray(inputs["w_own"], np.float32)
    w_int = np.asarray(inputs["w_int"], np.float32)
    wq = np.asarray(inputs["wq"], np.float32)
    wk = np.asarray(inputs["wk"], np.float32)
    wv = np.asarray(inputs["wv"], np.float32)
    v_att = np.asarray(inputs["v_att"], np.float32)
    w_attn = np.asarray(inputs["w_attn"], np.float32)
    w_ownp = np.asarray(inputs["w_ownp"], np.float32)
    w_h1 = np.asarray(inputs["w_h1"], np.float32)
    w_h2 = np.asarray(inputs["w_h2"], np.float32)
    w_out = np.asarray(inputs["w_out"], np.float32)

    def blockdiag128(w):  # [H, D, D] -> [128, 128]
        out = np.zeros((128, 128), np.float32)
        for h in range(H):
            out[h * D:(h + 1) * D, h * D:(h + 1) * D] = w[h]
        return out

    b_int = np.asarray(inputs["b_int"], np.float32)
    b_own = np.asarray(inputs["b_own"], np.float32)
    w2 = np.zeros((64, 128), np.float32)
    w2[0:7, 0:126] = w_int
    w2[7, 0:126] = b_int
    w2[32:39, 0:126] = w_int
    w2[39, 0:126] = b_int
    wn2 = np.zeros((64, 128), np.float32)
    wn2[0:7, 0:126] = w_own
    wn2[7, 0:126] = b_own

    va32 = np.zeros((128, 32), np.float32)
    for h in range(H):
        va32[h * D:(h + 1) * D, h] = v_att[h]

    ebt = np.zeros((128, 128), np.float32)
    for j in range(4):
        for h in range(H):
            ebt[32 * j + h, h * D:(h + 1) * D] = 1.0

    densel = np.zeros((128, 3), np.float32)
    for j in range(4):
        for h in range(H):
            densel[32 * j + h, h] = 1.0

    rbc = np.zeros((3, 128), np.float32)
    for h in range(H):
        rbc[h, h * D:(h + 1) * D] = 1.0

    wat = np.zeros((128, 128), np.float32)
    wat[0:126, :] = w_attn
    wop = np.zeros((128, 128), np.float32)
    wop[0:126, :] = w_ownp

    wh1r = np.ascontiguousarray(
        w_h1.reshape(2, 128, HID).transpose(1, 0, 2).reshape(128, 512))
    wh2r = np.ascontiguousarray(
        w_h2.reshape(2, 128, HID).transpose(1, 0, 2).reshape(128, 512))
    woutr = np.ascontiguousarray(
        w_out.reshape(2, 128, NOUT).transpose(1, 0, 2).reshape(128, 8))

    def pad_b(v, n=128):
        out = np.zeros((n, 1), np.float32)
        out[:v.shape[0], 0] = v
        return out

    params = {
        "w2": w2.astype(_bf16np), "wn2": wn2.astype(_bf16np),
        "wqb": blockdiag128(wq).astype(_bf16np),
        "wkb": blockdiag128(wk).astype(_bf16np),
        "wvb": blockdiag128(wv)[0:126].astype(_bf16np),
        "va32": va32.astype(_bf16np), "ebt": ebt.astype(_bf16np),
        "densel": densel.astype(_bf16np), "rbc": rbc.astype(_bf16np),
        "wat": wat.astype(_bf16np), "wop": wop.astype(_bf16np),
        "wh1r": wh1r.astype(_bf16np), "wh2r": wh2r.astype(_bf16np),
        "woutr": woutr.astype(_bf16np),
        "bown": pad_b(np.asarray(inputs["b_own"], np.float32)),
        "bint": pad_b(np.asarray(inputs["b_int"], np.float32)),
        "bat": pad_b(np.asarray(inputs["b_attn"], np.float32)),
        "bop": pad_b(np.asarray(inputs["b_ownp"], np.float32)),
        "bh1": np.ascontiguousarray(
            np.asarray(inputs["b_h1"], np.float32).reshape(2, 128).T),
        "bh2": np.ascontiguousarray(
            np.asarray(inputs["b_h2"], np.float32).reshape(2, 128).T),
        "bout": np.asarray(inputs["b_out"], np.float32).reshape(4, 1),
    }

    in_maps = []
    for c in range(N_CORES):
        sl = obs[c * BC:(c + 1) * BC]                        # [BC, 147]
        intr = sl[:, OWN_DIM:].reshape(BC, N_INTR, INT_DIM)  # [BC, 20, 7]
        intrT = intr.transpose(1, 2, 0)                      # [20, 7, BC]
        xp = np.zeros((64, NPAIR, BC), np.float32)
        xp[7, :, :] = 1.0
        xp[39, :, :] = 1.0
        for m in range(10):
            xp[0:7, m, :] = intrT[2 * m]
            xp[32:39, m, :] = intrT[2 * m + 1]
        xp[0:7, 10, :] = sl[:, :OWN_DIM].T
        m = {"xp": xp.astype(_bf16np)}
        m.update(params)
        in_maps.append(m)
    return in_maps


def _get_nc():
    if "nc" not in _BUILT:
        _BUILT["nc"] = _build_nc()
    return _BUILT["nc"]


def run(inputs, trace=False):
    from concourse.bass_utils import run_bass_kernel_spmd
    nc = _get_nc()
    in_maps = _host_prep(inputs)
    res = run_bass_kernel_spmd(nc, in_maps, core_ids=list(range(N_CORES)),
                               trace=trace)
    outs = [res.results[c]["outT"] for c in range(N_CORES)]   # each [4, BC]
    full = np.concatenate(outs, axis=1).T                     # [B, 4]
    return np.ascontiguousarray(full, dtype=np.float32), res


def kernel(**inputs):
    out, _ = run(inputs, trace=False)
    return out


# revision 39
# speedup vs baseline: 1.0368x; 1.0368x over previous
"""Trainium2 Bass kernel for nn_AttentionSACModel (sparse_attention).

Data-parallel across 8 NeuronCores: obs sharded along batch, params replicated.
Feature-major on-device layout (batch on the matmul free dim).

v2 structure (per 512-batch tile, per core):
  - z embeds: 2x row-tiled K=32 matmul pairs (x packed 2 interactors per
    64-partition slab), prelu batched over 2 PSUM banks.
  - energy: PK = Wk z + Wq own (2-pass accumulate), tanh batched over 2 banks.
  - scores: Va col-tiled 4x into one PSUM bank, exp per 4 interactors.
  - combine uses ctx = Wv (sum_n e_n z_n) * (1/denom) -- Wv applied once
    (blockdiag commutes with the per-head softmax weighting).
  - e-broadcast: row-tiled K=32 Eb matmul pairs; multiply paired [128,1024];
    sum over n via bf16 2x vector tree.
  - denom: Ds accumulate, reciprocal_approx_fast, Rb broadcast.
  - head MLP in bf16 weights.
"""
import sys
import os

if "/opt/trn_rl_repo" not in sys.path:
    sys.path.insert(0, "/opt/trn_rl_repo")

import numpy as np
import ml_dtypes
_bf16np = ml_dtypes.bfloat16

OWN_DIM = 7
INT_DIM = 7
N_INTR = 20
H = 3
D = 42
TOT = H * D            # 126
ATTN = 128
HID = 256
NOUT = 4
B = 32768
N_CORES = 8
BC = B // N_CORES      # 4096 rows per core
NB = 512               # batch tile (matmul free dim)
NT = BC // NB          # 8 tiles per core
ALPHA = 0.2            # leaky relu slope
NPAIR = 11             # 10 interactor pairs + (own, zero)
ZPRELU_V = (8, 9)  # z-pairs whose prelu runs off-scalar

_BUILT = {}


def _build_nc():
    import concourse.bacc as bacc
    import concourse.bass as bass
    import concourse.tile as tile
    from concourse import mybir

    f32 = mybir.dt.float32
    f32r = mybir.dt.float32r
    bf16 = mybir.dt.bfloat16
    AF = mybir.ActivationFunctionType
    ALU = mybir.AluOpType

    nc = bacc.Bacc()

    # ---- DRAM I/O ----
    xp_d = nc.dram_tensor("xp", [64, NPAIR, BC], bf16, kind="ExternalInput")
    w2_d = nc.dram_tensor("w2", [64, 128], bf16, kind="ExternalInput")     # W_int at rows 0-6 and 32-38
    wn2_d = nc.dram_tensor("wn2", [64, 128], bf16, kind="ExternalInput")   # W_own at rows 0-6
    wq_d = nc.dram_tensor("wqb", [128, 128], bf16, kind="ExternalInput")
    wk_d = nc.dram_tensor("wkb", [128, 128], bf16, kind="ExternalInput")
    wv_d = nc.dram_tensor("wvb", [126, 128], bf16, kind="ExternalInput")
    va_d = nc.dram_tensor("va32", [128, 32], bf16, kind="ExternalInput")
    eb_d = nc.dram_tensor("ebt", [128, 128], bf16, kind="ExternalInput")   # broadcast selector
    ds_d = nc.dram_tensor("densel", [128, 3], bf16, kind="ExternalInput")
    rb_d = nc.dram_tensor("rbc", [3, 128], bf16, kind="ExternalInput")
    wat_d = nc.dram_tensor("wat", [128, 128], bf16, kind="ExternalInput")
    wop_d = nc.dram_tensor("wop", [128, 128], bf16, kind="ExternalInput")
    wh1_d = nc.dram_tensor("wh1r", [128, 512], bf16, kind="ExternalInput")  # [p, kc*256+m]
    wh2_d = nc.dram_tensor("wh2r", [128, 512], bf16, kind="ExternalInput")
    wout_d = nc.dram_tensor("woutr", [128, 8], bf16, kind="ExternalInput")  # [p, kc*4+m]
    bown_d = nc.dram_tensor("bown", [128, 1], f32, kind="ExternalInput")
    bint_d = nc.dram_tensor("bint", [128, 1], f32, kind="ExternalInput")
    bat_d = nc.dram_tensor("bat", [128, 1], f32, kind="ExternalInput")
    bop_d = nc.dram_tensor("bop", [128, 1], f32, kind="ExternalInput")
    bh1_d = nc.dram_tensor("bh1", [128, 2], f32, kind="ExternalInput")
    bh2_d = nc.dram_tensor("bh2", [128, 2], f32, kind="ExternalInput")
    bout_d = nc.dram_tensor("bout", [4, 1], f32, kind="ExternalInput")
    out_d = nc.dram_tensor("outT", [NOUT, BC], f32, kind="ExternalOutput")

    with tile.TileContext(nc) as tc:
        with tc.tile_pool(name="const", bufs=1) as cst, \
             tc.tile_pool(name="px", bufs=3) as px, \
             tc.tile_pool(name="pzt", bufs=20) as pzt, \
             tc.tile_pool(name="pen", bufs=6) as pen, \
             tc.tile_pool(name="peg", bufs=12) as peg, \
             tc.tile_pool(name="ppn", bufs=21) as ppn, \
             tc.tile_pool(name="ptt", bufs=2) as ptt, \
             tc.tile_pool(name="ph", bufs=2) as ph, \
             tc.tile_pool(name="pwork", bufs=2, space="PSUM") as pwork, \
             tc.tile_pool(name="pps", bufs=1, space="PSUM") as pps, \
             tc.tile_pool(name="ppe", bufs=2, space="PSUM") as ppe, \
             tc.tile_pool(name="paux", bufs=1, space="PSUM") as paux:

            # ---- load constants ----
            W2 = cst.tile([64, 128], bf16)
            WN2 = cst.tile([64, 128], bf16)
            Wq = cst.tile([128, 128], bf16)
            Wk = cst.tile([128, 128], bf16)
            Wv = cst.tile([126, 128], bf16)
            Va = cst.tile([128, 32], bf16)
            EbT = cst.tile([128, 128], bf16)
            Ds = cst.tile([128, 3], bf16)
            Rb = cst.tile([3, 128], bf16)
            Wat = cst.tile([128, 128], bf16)
            Wop = cst.tile([128, 128], bf16)
            WH1 = cst.tile([128, 512], bf16)
            WH2 = cst.tile([128, 512], bf16)
            WOUT = cst.tile([128, 8], bf16)
            Bown = cst.tile([128, 1], f32)
            Bint = cst.tile([128, 1], f32)
            Bat = cst.tile([128, 1], f32)
            Bop = cst.tile([128, 1], f32)
            BH1 = cst.tile([128, 2], f32)
            BH2 = cst.tile([128, 2], f32)
            Bout = cst.tile([4, 1], f32)
            for t_sb, t_dr in [(W2, w2_d), (WN2, wn2_d), (Wk, wk_d),
                               (Wq, wq_d), (Bown, bown_d), (Bint, bint_d)]:
                nc.sync.dma_start(out=t_sb, in_=t_dr[:, :])

            def load_late_consts():
                for t_sb, t_dr in [(Va, va_d), (EbT, eb_d), (Ds, ds_d),
                                   (Rb, rb_d), (Wv, wv_d), (Wat, wat_d),
                                   (Wop, wop_d), (WH1, wh1_d), (WH2, wh2_d),
                                   (WOUT, wout_d), (Bat, bat_d), (Bop, bop_d),
                                   (BH1, bh1_d), (BH2, bh2_d), (Bout, bout_d)]:
                    nc.scalar.dma_start(out=t_sb, in_=t_dr[:, :])

            with nc.allow_low_precision(reason="bf16 intermediates; accums f32"):
                state = {}

                def load_x(t):
                    bs = t * NB
                    XP = px.tile([64, NPAIR, NB], bf16, tag="xp", name="XP")
                    nc.sync.dma_start(out=XP, in_=xp_d[:, :, bs:bs + NB])
                    state[t] = {"XP": XP, "ZT": [], "EG": []}

                def z_pair(t, m):
                    """embed pair m: 2x row-tiled K=32 matmuls + paired prelu"""
                    st = state[t]
                    XP = st["XP"]
                    PZ = pwork.tile([128, 2, NB], f32, tag="work", name="PZ")
                    lhs = WN2 if m == 10 else W2
                    nc.tensor.matmul(PZ[:, 0, :], lhs[0:32, :], XP[0:32, m, :],
                                     tile_position=(0, 0))
                    if m < 10:
                        nc.tensor.matmul(PZ[:, 1, :], lhs[32:64, :],
                                         XP[32:64, m, :], tile_position=(32, 0))
                    if m == 10:
                        OWN = ph.tile([128, NB], bf16, tag="own", name="OWN", bufs=4)
                        nc.scalar.activation(OWN, PZ[:, 0, :], AF.Prelu,
                                             alpha=ALPHA)
                        st["OWN"] = OWN
                    else:
                        ZT = pzt.tile([128, 2, NB], bf16, tag="zt", name="ZT")
                        if m in ZPRELU_V:
                            ZC = ptt.tile([128, 2, NB], bf16, tag="zc",
                                          name="ZC", bufs=3)
                            nc.vector.tensor_copy(ZC, PZ)
                            nc.vector.scalar_tensor_tensor(
                                out=ZT, in0=ZC, scalar=ALPHA, in1=ZC,
                                op0=ALU.mult, op1=ALU.max)
                        else:
                            nc.scalar.activation(ZT, PZ, AF.Prelu, alpha=ALPHA)
                        st["ZT"].append(ZT)

                def attn_energy(t, m):
                    """energy tanh for interactors 2m, 2m+1 of tile t"""
                    st = state[t]
                    OWN = st["OWN"]
                    PK = pwork.tile([128, 2, NB], f32, tag="work", name="PK")
                    ZT = st["ZT"][m]
                    # group same-weight matmuls so LDWEIGHTS amortizes
                    nc.tensor.matmul(PK[:, 0, :], Wk, ZT[:, 0, :],
                                     start=True, stop=False)
                    nc.tensor.matmul(PK[:, 1, :], Wk, ZT[:, 1, :],
                                     start=True, stop=False)
                    nc.tensor.matmul(PK[:, 0, :], Wq, OWN,
                                     start=False, stop=True)
                    nc.tensor.matmul(PK[:, 1, :], Wq, OWN,
                                     start=False, stop=True)
                    EN = pen.tile([128, 2, NB], bf16, tag="en", name="EN")
                    nc.scalar.activation(EN, PK, AF.Tanh)
                    st.setdefault("EN", []).append(EN)

                def attn_score(t, m):
                    """score matmul + exp for pair m (lagged so Va never
                    waits on the tanh)"""
                    st = state[t]
                    EN = st["EN"][m]
                    jj = (2 * m) % 4
                    if jj == 0:
                        st["PS"] = pps.tile([128, NB], f32, tag="ps", name="PS")
                    PS = st["PS"]
                    for i in range(2):
                        j = jj + i
                        nc.tensor.matmul(PS[32 * j:32 * (j + 1), :], Va, EN[:, i, :],
                                         tile_position=(0, 32 * j))
                    if jj == 2:
                        EG = peg.tile([128, NB], bf16, tag="eg", name="EG")
                        nc.scalar.activation(EG, PS, AF.Exp)
                        st["EG"].append(EG)

                def combine_one(t, n):
                    """broadcast exp (Eb matmul) + multiply with z"""
                    st = state[t]
                    g, j = n // 4, n % 4
                    EG = st["EG"][g]
                    PE1 = ppe.tile([128, NB], f32, tag="pe", name="PE1")
                    nc.tensor.matmul(PE1, EbT[32 * j:32 * (j + 1), :],
                                     EG[32 * j:32 * (j + 1), :],
                                     tile_position=(32 * j, 0))
                    PN = ppn.tile([126, NB], bf16, tag="pn", name="PN")
                    nc.vector.tensor_tensor(out=PN, in0=PE1[0:126, :],
                                            in1=st["ZT"][n // 2][0:126, n % 2, :],
                                            op=ALU.mult)
                    st.setdefault("PN", []).append(PN)

                def tt_add(st, key, a, b, tag, eng):
                    Tl = ptt.tile([126, NB], bf16, tag=tag, name=tag)
                    eng.tensor_tensor(out=Tl, in0=a, in1=b, op=ALU.add)
                    st[key] = Tl
                    return Tl

                def tree_l1(t, i):
                    """pair-add PN[2i] + PN[2i+1]; gpsimd helps (idle engine)"""
                    st = state[t]
                    PNs = st["PN"]
                    eng = nc.gpsimd if i % 2 == 0 else nc.vector
                    tt_add(st, f"T1_{i}", PNs[2 * i], PNs[2 * i + 1],
                           f"t1_{i}", eng)

                def tree_l2(t, j):
                    """T2_j = T1_{2j} + T1_{2j+1} (j = 0..4)"""
                    st = state[t]
                    eng = nc.gpsimd if j % 2 == 1 else nc.vector
                    tt_add(st, f"T2_{j}", st[f"T1_{2 * j}"],
                           st[f"T1_{2 * j + 1}"], f"t2_{j}", eng)

                def tree_finish(t, step):
                    st = state[t]
                    if step == 0:
                        tt_add(st, "T3A", st["T2_0"], st["T2_1"], "t3a",
                               nc.vector)
                    elif step == 1:
                        tt_add(st, "T3B", st["T2_2"], st["T2_3"], "t3b",
                               nc.gpsimd)
                    elif step == 2:
                        tt_add(st, "T4", st["T3A"], st["T3B"], "t4", nc.vector)
                    else:
                        U = ph.tile([126, NB], bf16, tag="u", name="U")
                        nc.vector.tensor_tensor(out=U, in0=st["T4"],
                                                in1=st["T2_4"], op=ALU.add)
                        st["U"] = U

                def denom_pd(t):
                    """softmax denominator -> broadcast reciprocal (no U dep)"""
                    st = state[t]
                    PD = paux.tile([128, NB], f32, tag="aux", name="PD")
                    for g in range(5):
                        nc.tensor.matmul(PD[0:3, :], Ds, st["EG"][g],
                                         start=(g == 0), stop=(g == 4))
                    RD = ph.tile([3, NB], f32, tag="rd", name="RD")
                    nc.vector.reciprocal_approx_fast(out=RD, in_=PD[0:3, :])
                    RDb = ph.tile([3, NB], bf16, tag="rdb", name="RDb")
                    nc.vector.tensor_copy(RDb, RD)
                    PR = paux.tile([128, NB], f32, tag="aux", name="PR")
                    nc.tensor.matmul(PR, Rb, RDb)
                    PRs = ph.tile([128, NB], bf16, tag="prs", name="PRs")
                    nc.vector.tensor_copy(PRs, PR)
                    st["PRs"] = PRs

                def denom_ctx(t):
                    """ctx = (Wv U) * (1/denom)"""
                    st = state[t]
                    PV = paux.tile([128, NB], f32, tag="aux", name="PV")
                    nc.tensor.matmul(PV, Wv, st["U"])
                    CTX = ph.tile([128, NB], bf16, tag="ctx", name="CTX")
                    nc.vector.tensor_tensor(out=CTX, in0=PV,
                                            in1=st["PRs"], op=ALU.mult)
                    st["CTX"] = CTX

                def head_steps(t):
                    """head MLP + output for tile t, as interleavable steps"""
                    bs = t * NB
                    h = {}

                    def s1():
                        PH1 = paux.tile([128, NB], f32, tag="aux", name="PH1")
                        nc.tensor.matmul(PH1, Wat, state[t]["CTX"])
                        h["ATT"] = ph.tile([128, NB], bf16, tag="att", name="ATT")
                        nc.scalar.activation(h["ATT"], PH1, AF.Tanh, bias=Bat)

                    def s2():
                        PH2 = paux.tile([128, NB], f32, tag="aux", name="PH2")
                        nc.tensor.matmul(PH2, Wop, state[t]["OWN"])
                        h["OWV"] = ph.tile([128, NB], bf16, tag="owv", name="OWV")
                        nc.scalar.activation(h["OWV"], PH2, AF.Tanh, bias=Bop)

                    def mk_h1(mh):
                        def s():
                            PHh = paux.tile([128, NB], f32, tag="aux", name="PHh")
                            nc.tensor.matmul(PHh, WH1[:, mh * 128:(mh + 1) * 128],
                                             h["OWV"], start=True, stop=False)
                            nc.tensor.matmul(PHh,
                                             WH1[:, 256 + mh * 128:256 + (mh + 1) * 128],
                                             h["ATT"], start=False, stop=True)
                            h[f"H1{mh}"] = ph.tile([128, NB], bf16,
                                                   tag=f"h1a{mh}", name="H1A")
                            nc.scalar.activation(h[f"H1{mh}"], PHh, AF.Prelu,
                                                 bias=BH1[:, mh:mh + 1], alpha=ALPHA)
                        return s

                    def mk_h2(mh):
                        def s():
                            PHh2 = paux.tile([128, NB], f32, tag="aux", name="PHh2")
                            nc.tensor.matmul(PHh2, WH2[:, mh * 128:(mh + 1) * 128],
                                             h["H10"], start=True, stop=False)
                            nc.tensor.matmul(PHh2,
                                             WH2[:, 256 + mh * 128:256 + (mh + 1) * 128],
                                             h["H11"], start=False, stop=True)
                            h[f"H2{mh}"] = ph.tile([128, NB], bf16,
                                                   tag=f"h2a{mh}", name="H2A")
                            nc.scalar.activation(h[f"H2{mh}"], PHh2, AF.Prelu,
                                                 bias=BH2[:, mh:mh + 1], alpha=ALPHA)
                        return s

                    def s7():
                        PO4 = paux.tile([128, NB], f32, tag="aux", name="PO4")
                        nc.tensor.matmul(PO4[0:4, :], WOUT[:, 0:4], h["H20"],
                                         start=True, stop=False)
                        nc.tensor.matmul(PO4[0:4, :], WOUT[:, 4:8], h["H21"],
                                         start=False, stop=True)
                        OT = ph.tile([4, NB], f32, tag="ot", name="OT")
                        nc.scalar.activation(OT, PO4[0:4, :], AF.Identity,
                                             bias=Bout)
                        nc.sync.dma_start(out=out_d[:, bs:bs + NB], in_=OT)
                        del state[t]

                    return [s1, s2, mk_h1(0), mk_h1(1), mk_h2(0), mk_h2(1), s7]

                def tile_body(t):
                    """z+attn for tile t, combine for t-1, head for t-2,
                    interleaved for engine-queue balance."""
                    prev = t - 1 if 0 <= t - 1 < NT else None
                    heads = head_steps(t - 2) if 0 <= t - 2 < NT else None
                    hi = 0

                    def do_head():
                        nonlocal hi
                        if heads is not None and hi < len(heads):
                            heads[hi]()
                            hi += 1

                    if t < NT and t + 1 < NT:
                        load_x(t + 1)
                    # fully merged schedule: every iteration feeds all four
                    # engines (z matmul + prelu for t, energy/score for t
                    # lagged, e-broadcast+mult+tree for t-1, head t-2)
                    for i in range(15):
                        if t < NT:
                            if i == 0:
                                z_pair(t, 10)        # own embed first
                            elif i <= 10:
                                z_pair(t, i - 1)
                        if prev is not None and i < 10:
                            combine_one(prev, 2 * i)
                            combine_one(prev, 2 * i + 1)
                        if t < NT and 4 <= i <= 13:
                            attn_energy(t, i - 4)
                        if t < NT and 5 <= i <= 14:
                            attn_score(t, i - 5)
                        if prev is not None:
                            if i == 1:
                                denom_pd(prev)
                            if i < 10:
                                tree_l1(prev, i)
                            if i in (3, 5, 7, 9, 11):
                                tree_l2(prev, (i - 3) // 2)
                            if i == 6:
                                tree_finish(prev, 0)
                            if i == 10:
                                tree_finish(prev, 1)
                            if i == 11:
                                tree_finish(prev, 2)
                            if i == 12:
                                tree_finish(prev, 3)
                                denom_ctx(prev)
                        if i in (2, 4, 6, 8, 10, 12):
                            do_head()
                    while heads is not None and hi < len(heads):
                        do_head()

                # software pipeline: t does z+attn; t-1 combine; t-2 head
                load_late_consts()
                load_x(0)
                for t in range(0, NT + 2):
                    tile_body(t)

    nc.compile()
    return nc


def _host_prep(inputs):
    """Build per-core input maps (numpy only)."""
    obs = np.ascontiguousarray(inputs["obs"], dtype=np.float32)
    w_own = np.asarray(inputs["w_own"], np.float32)
    w_int = np.asarray(inputs["w_int"], np.float32)
    wq = np.asarray(inputs["wq"], np.float32)
    wk = np.asarray(inputs["wk"], np.float32)
    wv = np.asarray(inputs["wv"], np.float32)
    v_att = np.asarray(inputs["v_att"], np.float32)
    w_attn = np.asarray(inputs["w_attn"], np.float32)
    w_ownp = np.asarray(inputs["w_ownp"], np.float32)
    w_h1 = np.asarray(inputs["w_h1"], np.float32)
    w_h2 = np.asarray(inputs["w_h2"], np.float32)
    w_out = np.asarray(inputs["w_out"], np.float32)

    def blockdiag128(w):  # [H, D, D] -> [128, 128]
        out = np.zeros((128, 128), np.float32)
        for h in range(H):
            out[h * D:(h + 1) * D, h * D:(h + 1) * D] = w[h]
        return out

    b_int = np.asarray(inputs["b_int"], np.float32)
    b_own = np.asarray(inputs["b_own"], np.float32)
    w2 = np.zeros((64, 128), np.float32)
    w2[0:7, 0:126] = w_int
    w2[7, 0:126] = b_int
    w2[32:39, 0:126] = w_int
    w2[39, 0:126] = b_int
    wn2 = np.zeros((64, 128), np.float32)
    wn2[0:7, 0:126] = w_own
    wn2[7, 0:126] = b_own

    va32 = np.zeros((128, 32), np.float32)
    for h in range(H):
        va32[h * D:(h + 1) * D, h] = v_att[h]

    ebt = np.zeros((128, 128), np.float32)
    for j in range(4):
        for h in range(H):
            ebt[32 * j + h, h * D:(h + 1) * D] = 1.0

    densel = np.zeros((128, 3), np.float32)
    for j in range(4):
        for h in range(H):
            densel[32 * j + h, h] = 1.0

    rbc = np.zeros((3, 128), np.float32)
    for h in range(H):
        rbc[h, h * D:(h + 1) * D] = 1.0

    wat = np.zeros((128, 128), np.float32)
    wat[0:126, :] = w_attn
    wop = np.zeros((128, 128), np.float32)
    wop[0:126, :] = w_ownp

    wh1r = np.ascontiguousarray(
        w_h1.reshape(2, 128, HID).transpose(1, 0, 2).reshape(128, 512))
    wh2r = np.ascontiguousarray(
        w_h2.reshape(2, 128, HID).transpose(1, 0, 2).reshape(128, 512))
    woutr = np.ascontiguousarray(
        w_out.reshape(2, 128, NOUT).transpose(1, 0, 2).reshape(128, 8))

    def pad_b(v, n=128):
        out = np.zeros((n, 1), np.float32)
        out[:v.shape[0], 0] = v
        return out

    params = {
        "w2": w2.astype(_bf16np), "wn2": wn2.astype(_bf16np),
        "wqb": blockdiag128(wq).astype(_bf16np),
        "wkb": blockdiag128(wk).astype(_bf16np),
        "wvb": blockdiag128(wv)[0:126].astype(_bf16np),
        "va32": va32.astype(_bf16np), "ebt": ebt.astype(_bf16np),
        "densel": densel.astype(_bf16np), "rbc": rbc.astype(_bf16np),
        "wat": wat.astype(_bf16np), "wop": wop.astype(_bf16np),
        "wh1r": wh1r.astype(_bf16np), "wh2r": wh2r.astype(_bf16np),
        "woutr": woutr.astype(_bf16np),
        "bown": pad_b(np.asarray(inputs["b_own"], np.float32)),
        "bint": pad_b(np.asarray(inputs["b_int"], np.float32)),
        "bat": pad_b(np.asarray(inputs["b_attn"], np.float32)),
        "bop": pad_b(np.asarray(inputs["b_ownp"], np.float32)),
        "bh1": np.ascontiguousarray(
            np.asarray(inputs["b_h1"], np.float32).reshape(2, 128).T),
        "bh2": np.ascontiguousarray(
            np.asarray(inputs["b_h2"], np.float32).reshape(2, 128).T),
        "bout": np.asarray(inputs["b_out"], np.float32).reshape(4, 1),
    }

    in_maps = []
    for c in range(N_CORES):
        sl = obs[c * BC:(c + 1) * BC]                        # [BC, 147]
        intr = sl[:, OWN_DIM:].reshape(BC, N_INTR, INT_DIM)  # [BC, 20, 7]
        intrT = intr.transpose(1, 2, 0)                      # [20, 7, BC]
        xp = np.zeros((64, NPAIR, BC), np.float32)
        xp[7, :, :] = 1.0
        xp[39, :, :] = 1.0
        for m in range(10):
            xp[0:7, m, :] = intrT[2 * m]
            xp[32:39, m, :] = intrT[2 * m + 1]
        xp[0:7, 10, :] = sl[:, :OWN_DIM].T
        m = {"xp": xp.astype(_bf16np)}
        m.update(params)
        in_maps.append(m)
    return in_maps


def _get_nc():
    if "nc" not in _BUILT:
        _BUILT["nc"] = _build_nc()
    return _BUILT["nc"]


def run(inputs, trace=False):
    from concourse.bass_utils import run_bass_kernel_spmd
    nc = _get_nc()
    in_maps = _host_prep(inputs)
    res = run_bass_kernel_spmd(nc, in_maps, core_ids=list(range(N_CORES)),
                               trace=trace)
    outs = [res.results[c]["outT"] for c in range(N_CORES)]   # each [4, BC]
    full = np.concatenate(outs, axis=1).T                     # [B, 4]
    return np.ascontiguousarray(full, dtype=np.float32), res


def kernel(**inputs):
    out, _ = run(inputs, trace=False)
    return out
